# revision 34
# baseline (speedup 1.0000x reference)
"""Bass/Trainium2 kernel for nn_F_Loss_65446711656630.

Strategy (data-parallel over N, 8 cores):
  - Host: GLOBAL stable sort of all rows by class id, quantize to fp8 e4m3,
    then lay out per-core operands:
      * hta (features 0-383, row-major + interleaved ones cols): PE path.
      * htb (features 384-511, feature-major): DVE/ACT path.
  - Device, work split by measured engine rates. The PE's HAM clock gate
    runs the array at 1.2 GHz until ~3.4us of sustained activity, then
    2.4 GHz (59 ns per 129-col fp8 matmul). So:
      * ~26 warm-up matmuls on a zeroed dummy tile run during the DMA
        trigger phase, putting the PE in the warm state by the time real
        data lands, and the DMA order keeps the PE stream gap-free
        (gaps > 3.4us re-throttle it).
      * TensorE (12/16 granule-blocks): per 128-row chunk, one fp8 matmul
        per block with stationary = X_b, moving = [X_b | 1] accumulates
        X_b^T X_b (diag = sumsq) and X_b^T 1 (sums) into PSUM.
      * DVE (3/16): bn_stats (count/mean/count*var per 512-elem subtile,
        the HW max) gives BOTH stats in one pass at ~1.19 ns/elem.
      * ACT (1/16): Square+accum_out and Copy+accum_out passes, plus the
        PSUM->SBUF bf16 stage copies (DMA cannot read PSUM directly).
      * DMA triggers cost ~620ns of serial queue time each; they are
        placed on the Sync queue in consumption order (PE data first),
        with the first DVE/ACT tile on the ACT queue in parallel.
  - Host: per-class stats from single-class granule partials (f64) +
    direct numpy f64 sums for class-transition granules; then the tiny
    O(C^2 D) pairwise betainc/top-k stage in f32 jax on CPU (mirroring
    the reference's numerics exactly).
"""

import os

import ml_dtypes
import numpy as np

# safety net: recover cleanly if a previous process left a NeuronCore wedged
os.environ.setdefault("NEURON_RT_RESET_CORES", "1")

C = 16
D = 512
N = 65536
NCORES = 8
ROWS = N // NCORES          # 8192 rows per core
P = 128                     # SBUF partitions
GRAN = 2048                 # rows per granule (stats accumulation unit)
NGRAN = ROWS // GRAN        # 4 granules per core
NCHK = GRAN // P            # 16 chunks per granule
HCHK = NCHK // 2            # 8 chunks per half-granule
PEBLK = 3                   # feature blocks on the PE
BCOL = P + 1                # 129 cols per PE block: 128 features + ones col
NDUMMY = 26                 # HAM warm-up matmuls
XMIN, XMAX = 1e-37, 1.0 - 1e-5

ACC_COLS = 124              # DVE bn stats + ACT accums

F8 = ml_dtypes.float8_e4m3

_NC_CACHE = {}


def _build_nc():
    """Per-core SPMD program.

    Inputs:  "hta"  [4, 128, 2, 8, 387] fp8e4  (granule, partition, half,
               chunk, 3 blocks x [128 feat | 1.0]; row r within granule =
               (half*8 + chunk)*128 + p)
             "htb"  [128, 8, 1024] fp8e4  (block 3 feature-major halves,
               granule-major: g0h0, g0h1, g1h0, ... g3h1)
    Outputs: "peo"  [128, 12, 129] bf16  (Gram stats, col 3g+b:
               peo[f, 3g+b, c] = sum over granule g of
               X[:, b*128+f] * X[:, b*128+c] for c<128, sum of
               X[:, b*128+f] at c==128)
             "acc"  [128, 124] f32  (cols 24(g-1)..24(g-1)+23: bn_stats of
               block 3 granule g (g>=1) as [4 subtiles x 6]; col 120/121:
               ACT sumsq/sum of block 3 granule 0)
    """
    import concourse.tile as tile
    from concourse import bacc, mybir

    f32 = mybir.dt.float32
    bf16 = mybir.dt.bfloat16
    f8 = mybir.dt.float8e4

    nc = bacc.Bacc("TRN2", target_bir_lowering=False, debug=False,
                   num_devices=NCORES)
    hta = nc.declare_dram_parameter("hta", [NGRAN, P, 2, HCHK, PEBLK * BCOL],
                                    f8, isOutput=False)
    htb = nc.declare_dram_parameter("htb", [P, 8, 1024], f8, isOutput=False)
    peo = nc.declare_dram_parameter("peo", [P, PEBLK * NGRAN, BCOL], bf16,
                                    isOutput=True)
    accp = nc.declare_dram_parameter("acc", [P, ACC_COLS], f32, isOutput=True)

    with tile.TileContext(nc) as tc:
        with (
            tc.tile_pool(name="in", bufs=1) as in_pool,
            tc.tile_pool(name="sc", bufs=2) as scr_pool,
            tc.tile_pool(name="st", bufs=1) as stg_pool,
            tc.tile_pool(name="ps", bufs=2, space="PSUM") as psum_pool,
            tc.tile_pool(name="pd", bufs=1, space="PSUM") as pdum_pool,
        ):
            acc_t = stg_pool.tile([P, ACC_COLS], f32, tag="acc")
            so = stg_pool.tile([P, PEBLK * NGRAN, BCOL], bf16, tag="so")
            dmy = stg_pool.tile([P, BCOL], f8, tag="dmy")
            nc.gpsimd.memset(dmy[:], 0.0)

            # ---- input DMAs ------------------------------------------
            # Each HWDGE engine owns exactly ONE hardware dynamic queue
            # (~200 GB/s each), so the input bytes are split ~50/50
            # between the Sync and ACT queues, in consumption order, with
            # hta granules split in halves across the two queues.
            ta = [in_pool.tile([P, 2, HCHK, PEBLK * BCOL], f8, tag=f"ta{g}",
                               name=f"ta{g}") for g in range(NGRAN)]
            tbt = [in_pool.tile([P, 2, 1024], f8, tag=f"tb{g}",
                                name=f"tb{g}") for g in range(NGRAN)]
            # Sync queue
            nc.sync.dma_start(ta[0][:, 0], hta[0][:, 0])
            nc.sync.dma_start(ta[0][:, 1], hta[0][:, 1])
            nc.sync.dma_start(ta[1][:, 0], hta[1][:, 0])
            nc.sync.dma_start(ta[2][:, 0], hta[2][:, 0])
            nc.sync.dma_start(tbt[3][:], htb[:, 6:8])
            nc.sync.dma_start(ta[3][:, 0], hta[3][:, 0])
            # ACT queue (tb1 ahead of ta1h1: DVE's 8.2us bn chain is
            # tail-critical and must start as early as possible)
            nc.scalar.dma_start(tbt[0][:], htb[:, 0:2])
            nc.scalar.dma_start(tbt[1][:], htb[:, 2:4])
            nc.scalar.dma_start(ta[1][:, 1], hta[1][:, 1])
            nc.scalar.dma_start(tbt[2][:], htb[:, 4:6])
            nc.scalar.dma_start(ta[2][:, 1], hta[2][:, 1])
            nc.scalar.dma_start(ta[3][:, 1], hta[3][:, 1])
            tb0, tb1, tb2, tb3 = tbt

            # ---- TensorE: HAM warm-up, then Gram blocks 0-2 -----------
            pdt = pdum_pool.tile([P, BCOL], f32, tag="pdt")
            for _ in range(NDUMMY):
                nc.tensor.matmul(pdt[:], dmy[:, 0:P], dmy[:, 0:BCOL],
                                 start=True, stop=True)

            pts = []
            for g in range(NGRAN):
                # one PSUM bank per block: matmul output regions must be
                # bank-aligned (packing 3x129 into one bank corrupts the
                # non-aligned blocks)
                pt = psum_pool.tile([P, PEBLK, 512], f32, tag="ps",
                                    name="pt")
                # half-major (in data-arrival order), then block-major:
                # runs of 8 matmuls accumulate into the same PSUM region
                # (pipelined drains, ~57ns/MM warm) and a granule's work
                # can start when its first half lands
                horder = (0, 1) if g % 2 == 0 else (1, 0)
                for hi, h in enumerate(horder):
                    for b in range(PEBLK):
                        for lc in range(HCHK):
                            th = ta[g][:, h, lc]
                            nc.tensor.matmul(
                                pt[:, b, 0:BCOL],
                                th[:, b * BCOL:b * BCOL + P],
                                th[:, b * BCOL:b * BCOL + BCOL],
                                start=(hi == 0 and lc == 0),
                                stop=(hi == 1 and lc == HCHK - 1))
                pts.append(pt)

            # ---- DVE: bn_stats for block 3, granules 1-3 --------------
            # (hardware caps BN_STATS at 512 elements per instruction)
            def bn(dst_col, src_ap):
                flat = src_ap.rearrange("p a (b x) -> p (a b) x", x=512)
                for i in range(4):
                    nc.vector.bn_stats(
                        acc_t[:, dst_col + 6 * i:dst_col + 6 * i + 6],
                        flat[:, i])

            bn(0, tb1[:])
            bn(24, tb2[:])
            bn(48, tb3[:])
            # granule 3's stage copy on DVE: it is idle by then, while ACT
            # would gate the final output DMA
            nc.vector.tensor_copy(so[:, PEBLK * 3:], pts[3][:, :, 0:BCOL])

            # ---- ACT: block 3 granule 0 + all PSUM stage copies -------
            scr = scr_pool.tile([P, 2, 1024], bf16, tag="scr")
            nc.scalar.activation(
                scr[:], tb0[:], mybir.ActivationFunctionType.Square,
                accum_out=acc_t[:, 120:121])
            scr2 = scr_pool.tile([P, 2, 1024], bf16, tag="scr2")
            nc.scalar.activation(
                scr2[:], tb0[:], mybir.ActivationFunctionType.Copy,
                accum_out=acc_t[:, 121:122])
            for g in range(3):
                nc.scalar.copy(so[:, PEBLK * g:PEBLK * (g + 1)],
                               pts[g][:, :, 0:BCOL])

            # ---- output DMAs (final pieces on the emptier ACT queue) ---
            nc.sync.dma_start(peo[:, 0:PEBLK * 3], so[:, 0:PEBLK * 3])
            nc.scalar.dma_start(accp[:], acc_t[:])
            nc.scalar.dma_start(peo[:, PEBLK * 3:], so[:, PEBLK * 3:])
    nc.compile()
    return nc


def _get_nc():
    if "nc" not in _NC_CACHE:
        _NC_CACHE["nc"] = _build_nc()
    return _NC_CACHE["nc"]


def _granule_classes(ids_sorted, size):
    """Per-granule class id, or -1 if the granule spans a class boundary."""
    g = ids_sorted.reshape(-1, size)
    pure = g[:, 0] == g[:, -1]
    return np.where(pure, g[:, 0], -1).astype(np.int64)


def _prep_core(hs_k, ids_k):
    """hs_k/ids_k already globally sorted. Returns device inputs + host fixups."""
    q = hs_k.astype(F8)

    # hta: features 0-383, row-major with interleaved ones columns
    q5 = q[:, :PEBLK * P].reshape(NGRAN, NCHK, P, PEBLK, P)
    buf = np.empty((NGRAN, P, NCHK, PEBLK, BCOL), dtype=F8)
    buf[..., :P] = q5.transpose(0, 2, 1, 3, 4)
    buf[..., P] = np.array(1.0, dtype=F8)
    hta = buf.reshape(NGRAN, P, 2, HCHK, PEBLK * BCOL)

    # htb: block 3 feature-major halves, granule-major
    htb = q[:, PEBLK * P:].reshape(NGRAN * 2, 1024, P).transpose(2, 0, 1)
    htb = np.ascontiguousarray(htb)

    gcls = _granule_classes(ids_k, GRAN)          # [4]

    bsum = np.zeros((C, D), dtype=np.float64)
    bsq = np.zeros((C, D), dtype=np.float64)
    # transition granules: host computes their per-class stats exactly
    if (gcls < 0).any():
        m = np.repeat(gcls < 0, GRAN)
        rows, rids = hs_k[m].astype(np.float64), ids_k[m]
        for cq in np.unique(rids):
            sel = rows[rids == cq]
            bsum[cq] += sel.sum(axis=0)
            bsq[cq] += (sel * sel).sum(axis=0)
    return {"hta": hta, "htb": htb}, gcls, bsum, bsq


def _decode_bn(block):
    """block: [128, 4, 6] f64 -> (sums[128], sumsq[128])."""
    ce, me, ve = block[:, :, 0], block[:, :, 1], block[:, :, 2]
    co, mo, vo = block[:, :, 3], block[:, :, 4], block[:, :, 5]
    sums = (ce * me + co * mo).sum(axis=1)
    sumsq = (ve + ce * me * me + vo + co * mo * mo).sum(axis=1)
    return sums, sumsq


def _device_stats(hidden, ids, **run_kwargs):
    """Returns (sums[C,D], sumsq[C,D]) float64, plus the raw run result."""
    from concourse import bass_utils

    nc = _get_nc()

    order = np.argsort(ids, kind="stable")       # GLOBAL sort by class
    ids_s = ids[order]
    hs = hidden[order]

    in_maps = []
    meta = []
    sums = np.zeros((C, D), dtype=np.float64)
    sumsq = np.zeros((C, D), dtype=np.float64)
    for k in range(NCORES):
        rows = slice(k * ROWS, (k + 1) * ROWS)
        im, gcls, bsum, bsq = _prep_core(hs[rows], ids_s[rows])
        in_maps.append(im)
        meta.append(gcls)
        sums += bsum
        sumsq += bsq

    res = bass_utils.run_bass_kernel_spmd(nc, in_maps, list(range(NCORES)),
                                          **run_kwargs)

    DPE = PEBLK * P  # 384 features on the PE path
    for k in range(NCORES):
        gcls = meta[k]
        peo = res.results[k]["peo"].astype(np.float64)   # [128, 12, 129]
        st = peo.reshape(P, NGRAN, PEBLK, BCOL).transpose(1, 0, 2, 3)
        acc = res.results[k]["acc"].astype(np.float64)   # [128, 124]

        gsums = np.empty((NGRAN, D))
        gsq = np.empty((NGRAN, D))
        # features 0-383 from PE Gram blocks (feature = b*128 + f)
        gsums[:, :DPE] = st[:, :, :, P].transpose(0, 2, 1).reshape(NGRAN, DPE)
        gsq[:, :DPE] = np.diagonal(st[:, :, :, :P], axis1=1,
                                   axis2=3).reshape(NGRAN, DPE)
        # features 384-511: ACT accums for g0, DVE bn for g1-g3
        gsq[0, DPE:] = acc[:, 120]
        gsums[0, DPE:] = acc[:, 121]
        for g in range(1, NGRAN):
            s, sq = _decode_bn(acc[:, 24 * (g - 1):24 * (g - 1) + 24]
                               .reshape(P, 4, 6))
            gsums[g, DPE:] = s
            gsq[g, DPE:] = sq

        for g in range(NGRAN):
            cc = gcls[g]
            if cc >= 0:
                sums[cc] += gsums[g]
                sumsq[cc] += gsq[g]
    return sums, sumsq, res


def _pairwise_loss(counts, sums, sumsq, d):
    """The tiny O(C^2 D) stage on host CPU.

    Runs in float32 with the same jax ops as the reference: at these extreme
    betainc parameters (b ~ 8190, x ~ 1e-5) jax's f32 betainc differs from
    the true (f64) value by ~1e-3, so matching the reference requires
    replicating its f32 numerics, not improving on them.
    """
    import jax
    import jax.numpy as jnp

    cpu = jax.devices("cpu")[0]
    with jax.default_device(cpu):
        counts64 = counts.astype(np.float64)
        means64 = sums / counts64[:, None]
        withins64 = sumsq - counts64[:, None] * means64**2
        counts = jnp.asarray(counts64, jnp.float32)               # [C]
        means = jnp.asarray(means64, jnp.float32)                 # [C, D]
        withins = jnp.asarray(withins64, jnp.float32)             # [C, D]
        half_diff = (means[:, None, :] - means[None, :, :]) * 0.5
        pair_counts = counts[:, None] + counts[None, :]
        pair_between = half_diff * half_diff * pair_counts[:, :, None]
        pair_within = withins[:, None, :] + withins[None, :, :]
        d2 = pair_counts - 2.0
        d2 = jnp.where(d2 == 0.0, 1e-5, d2)
        x = pair_between / (pair_between + pair_within)
        x = jnp.clip(x, XMIN, XMAX)
        a = jnp.full_like(x, 0.5)
        b = jnp.broadcast_to((d2 * 0.5)[:, :, None], x.shape)
        xbetainc = jax.scipy.special.betainc(a, b, x)             # [C, C, D]
        top_k, _ = jax.lax.top_k(xbetainc, int(d))                # [C, C, d]
        per_pair = jnp.sum(jnp.log(top_k), axis=-1)               # [C, C]
        mask = jnp.triu(jnp.ones((C, C), dtype=bool), k=1)
        total = jnp.sum(jnp.where(mask, per_pair, jnp.zeros_like(per_pair)))
        return float(-total)


def kernel(hidden, batch_ids, d):
    hidden = np.asarray(hidden, dtype=np.float32)
    ids = np.asarray(batch_ids).astype(np.int64)
    assert hidden.shape == (N, D), hidden.shape

    counts = np.bincount(ids, minlength=C).astype(np.float64)
    sums, sumsq, _ = _device_stats(hidden, ids)
    total = _pairwise_loss(counts, sums, sumsq, int(np.asarray(d)))
    return np.array(total, dtype=np.float32)


# revision 35
# speedup vs baseline: 1.0147x; 1.0147x over previous
"""Bass/Trainium2 kernel for nn_F_Loss_65446711656630.

Strategy (data-parallel over N, 8 cores):
  - Host: GLOBAL stable sort of all rows by class id, quantize to fp8 e4m3,
    then lay out per-core operands:
      * hta (features 0-383, row-major + interleaved ones cols): PE path.
      * htb (features 384-511, feature-major): DVE/ACT path.
  - Device, work split by measured engine rates. The PE's HAM clock gate
    runs the array at 1.2 GHz until ~3.4us of sustained activity, then
    2.4 GHz (59 ns per 129-col fp8 matmul). So:
      * ~26 warm-up matmuls on a zeroed dummy tile run during the DMA
        trigger phase, putting the PE in the warm state by the time real
        data lands, and the DMA order keeps the PE stream gap-free
        (gaps > 3.4us re-throttle it).
      * TensorE (12/16 granule-blocks): per 128-row chunk, one fp8 matmul
        per block with stationary = X_b, moving = [X_b | 1] accumulates
        X_b^T X_b (diag = sumsq) and X_b^T 1 (sums) into PSUM.
      * DVE (3/16): bn_stats (count/mean/count*var per 512-elem subtile,
        the HW max) gives BOTH stats in one pass at ~1.19 ns/elem.
      * ACT (1/16): Square+accum_out and Copy+accum_out passes, plus the
        PSUM->SBUF bf16 stage copies (DMA cannot read PSUM directly).
      * DMA triggers cost ~620ns of serial queue time each; they are
        placed on the Sync queue in consumption order (PE data first),
        with the first DVE/ACT tile on the ACT queue in parallel.
  - Host: per-class stats from single-class granule partials (f64) +
    direct numpy f64 sums for class-transition granules; then the tiny
    O(C^2 D) pairwise betainc/top-k stage in f32 jax on CPU (mirroring
    the reference's numerics exactly).
"""

import os

import ml_dtypes
import numpy as np

# safety net: recover cleanly if a previous process left a NeuronCore wedged
os.environ.setdefault("NEURON_RT_RESET_CORES", "1")

C = 16
D = 512
N = 65536
NCORES = 8
ROWS = N // NCORES          # 8192 rows per core
P = 128                     # SBUF partitions
GRAN = 2048                 # rows per granule (stats accumulation unit)
NGRAN = ROWS // GRAN        # 4 granules per core
NCHK = GRAN // P            # 16 chunks per granule
HCHK = NCHK // 2            # 8 chunks per half-granule
PEBLK = 3                   # feature blocks on the PE
BCOL = P + 1                # 129 cols per PE block: 128 features + ones col
NDUMMY = 40                 # HAM warm-up matmuls (bridge until real data
                            # lands: a PE-idle gap resets the busy window)
XMIN, XMAX = 1e-37, 1.0 - 1e-5

ACC_COLS = 124              # DVE bn stats + ACT accums

F8 = ml_dtypes.float8_e4m3

_NC_CACHE = {}


def _build_nc():
    """Per-core SPMD program.

    Inputs:  "hta"  [4, 128, 2, 8, 387] fp8e4  (granule, partition, half,
               chunk, 3 blocks x [128 feat | 1.0]; row r within granule =
               (half*8 + chunk)*128 + p)
             "htb"  [128, 8, 1024] fp8e4  (block 3 feature-major halves,
               granule-major: g0h0, g0h1, g1h0, ... g3h1)
    Outputs: "peo"  [128, 12, 129] bf16  (Gram stats, col 3g+b:
               peo[f, 3g+b, c] = sum over granule g of
               X[:, b*128+f] * X[:, b*128+c] for c<128, sum of
               X[:, b*128+f] at c==128)
             "acc"  [128, 124] f32  (cols 24(g-1)..24(g-1)+23: bn_stats of
               block 3 granule g (g>=1) as [4 subtiles x 6]; col 120/121:
               ACT sumsq/sum of block 3 granule 0)
    """
    import concourse.tile as tile
    from concourse import bacc, mybir

    f32 = mybir.dt.float32
    bf16 = mybir.dt.bfloat16
    f8 = mybir.dt.float8e4

    nc = bacc.Bacc("TRN2", target_bir_lowering=False, debug=False,
                   num_devices=NCORES)
    hta = nc.declare_dram_parameter("hta", [NGRAN, P, 2, HCHK, PEBLK * BCOL],
                                    f8, isOutput=False)
    htb = nc.declare_dram_parameter("htb", [P, 8, 1024], f8, isOutput=False)
    peo = nc.declare_dram_parameter("peo", [P, PEBLK * NGRAN, BCOL], bf16,
                                    isOutput=True)
    accp = nc.declare_dram_parameter("acc", [P, ACC_COLS], f32, isOutput=True)

    with tile.TileContext(nc) as tc:
        with (
            tc.tile_pool(name="in", bufs=1) as in_pool,
            tc.tile_pool(name="sc", bufs=2) as scr_pool,
            tc.tile_pool(name="st", bufs=1) as stg_pool,
            tc.tile_pool(name="ps", bufs=2, space="PSUM") as psum_pool,
            tc.tile_pool(name="pd", bufs=1, space="PSUM") as pdum_pool,
        ):
            acc_t = stg_pool.tile([P, ACC_COLS], f32, tag="acc")
            so = stg_pool.tile([P, PEBLK * NGRAN, BCOL], bf16, tag="so")
            dmy = stg_pool.tile([P, BCOL], f8, tag="dmy")
            nc.gpsimd.memset(dmy[:], 0.0)

            # ---- input DMAs ------------------------------------------
            # Each HWDGE engine owns exactly ONE hardware dynamic queue
            # (~200 GB/s each), so the input bytes are split ~50/50
            # between the Sync and ACT queues, in consumption order, with
            # hta granules split in halves across the two queues.
            ta = [in_pool.tile([P, 2, HCHK, PEBLK * BCOL], f8, tag=f"ta{g}",
                               name=f"ta{g}") for g in range(NGRAN)]
            tbt = [in_pool.tile([P, 2, 1024], f8, tag=f"tb{g}",
                                name=f"tb{g}") for g in range(NGRAN)]
            # Sync queue
            nc.sync.dma_start(ta[0][:, 0], hta[0][:, 0])
            nc.sync.dma_start(ta[0][:, 1], hta[0][:, 1])
            nc.sync.dma_start(ta[1][:, 0], hta[1][:, 0])
            nc.sync.dma_start(ta[2][:, 0], hta[2][:, 0])
            nc.sync.dma_start(tbt[3][:], htb[:, 6:8])
            nc.sync.dma_start(ta[3][:, 0], hta[3][:, 0])
            # ACT queue (tb1 ahead of ta1h1: DVE's 8.2us bn chain is
            # tail-critical and must start as early as possible)
            nc.scalar.dma_start(tbt[0][:], htb[:, 0:2])
            nc.scalar.dma_start(tbt[1][:], htb[:, 2:4])
            nc.scalar.dma_start(ta[1][:, 1], hta[1][:, 1])
            nc.scalar.dma_start(tbt[2][:], htb[:, 4:6])
            nc.scalar.dma_start(ta[2][:, 1], hta[2][:, 1])
            nc.scalar.dma_start(ta[3][:, 1], hta[3][:, 1])
            tb0, tb1, tb2, tb3 = tbt

            # ---- TensorE: HAM warm-up, then Gram blocks 0-2 -----------
            pdt = pdum_pool.tile([P, BCOL], f32, tag="pdt")
            for _ in range(NDUMMY):
                nc.tensor.matmul(pdt[:], dmy[:, 0:P], dmy[:, 0:BCOL],
                                 start=True, stop=True)

            pts = []
            for g in range(NGRAN):
                # one PSUM bank per block: matmul output regions must be
                # bank-aligned (packing 3x129 into one bank corrupts the
                # non-aligned blocks)
                pt = psum_pool.tile([P, PEBLK, 512], f32, tag="ps",
                                    name="pt")
                # half-major (in data-arrival order), then block-major:
                # runs of 8 matmuls accumulate into the same PSUM region
                # (pipelined drains, ~57ns/MM warm) and a granule's work
                # can start when its first half lands
                horder = (0, 1) if g % 2 == 0 else (1, 0)
                for hi, h in enumerate(horder):
                    for b in range(PEBLK):
                        for lc in range(HCHK):
                            th = ta[g][:, h, lc]
                            nc.tensor.matmul(
                                pt[:, b, 0:BCOL],
                                th[:, b * BCOL:b * BCOL + P],
                                th[:, b * BCOL:b * BCOL + BCOL],
                                start=(hi == 0 and lc == 0),
                                stop=(hi == 1 and lc == HCHK - 1))
                pts.append(pt)

            # ---- DVE: bn_stats for block 3, granules 1-3 --------------
            # (hardware caps BN_STATS at 512 elements per instruction)
            def bn(dst_col, src_ap):
                flat = src_ap.rearrange("p a (b x) -> p (a b) x", x=512)
                for i in range(4):
                    nc.vector.bn_stats(
                        acc_t[:, dst_col + 6 * i:dst_col + 6 * i + 6],
                        flat[:, i])

            bn(0, tb1[:])
            bn(24, tb2[:])
            bn(48, tb3[:])
            # granule 3's stage copy on DVE: it is idle by then, while ACT
            # would gate the final output DMA
            nc.vector.tensor_copy(so[:, PEBLK * 3:], pts[3][:, :, 0:BCOL])

            # ---- ACT: block 3 granule 0 + all PSUM stage copies -------
            scr = scr_pool.tile([P, 2, 1024], bf16, tag="scr")
            nc.scalar.activation(
                scr[:], tb0[:], mybir.ActivationFunctionType.Square,
                accum_out=acc_t[:, 120:121])
            scr2 = scr_pool.tile([P, 2, 1024], bf16, tag="scr2")
            nc.scalar.activation(
                scr2[:], tb0[:], mybir.ActivationFunctionType.Copy,
                accum_out=acc_t[:, 121:122])
            for g in range(3):
                nc.scalar.copy(so[:, PEBLK * g:PEBLK * (g + 1)],
                               pts[g][:, :, 0:BCOL])

            # ---- output DMAs (final pieces on the emptier ACT queue) ---
            nc.sync.dma_start(peo[:, 0:PEBLK * 3], so[:, 0:PEBLK * 3])
            nc.scalar.dma_start(accp[:], acc_t[:])
            nc.scalar.dma_start(peo[:, PEBLK * 3:], so[:, PEBLK * 3:])
    nc.compile()
    return nc


def _get_nc():
    if "nc" not in _NC_CACHE:
        _NC_CACHE["nc"] = _build_nc()
    return _NC_CACHE["nc"]


def _granule_classes(ids_sorted, size):
    """Per-granule class id, or -1 if the granule spans a class boundary."""
    g = ids_sorted.reshape(-1, size)
    pure = g[:, 0] == g[:, -1]
    return np.where(pure, g[:, 0], -1).astype(np.int64)


def _prep_core(hs_k, ids_k):
    """hs_k/ids_k already globally sorted. Returns device inputs + host fixups."""
    q = hs_k.astype(F8)

    # hta: features 0-383, row-major with interleaved ones columns
    q5 = q[:, :PEBLK * P].reshape(NGRAN, NCHK, P, PEBLK, P)
    buf = np.empty((NGRAN, P, NCHK, PEBLK, BCOL), dtype=F8)
    buf[..., :P] = q5.transpose(0, 2, 1, 3, 4)
    buf[..., P] = np.array(1.0, dtype=F8)
    hta = buf.reshape(NGRAN, P, 2, HCHK, PEBLK * BCOL)

    # htb: block 3 feature-major halves, granule-major
    htb = q[:, PEBLK * P:].reshape(NGRAN * 2, 1024, P).transpose(2, 0, 1)
    htb = np.ascontiguousarray(htb)

    gcls = _granule_classes(ids_k, GRAN)          # [4]

    bsum = np.zeros((C, D), dtype=np.float64)
    bsq = np.zeros((C, D), dtype=np.float64)
    # transition granules: host computes their per-class stats exactly
    if (gcls < 0).any():
        m = np.repeat(gcls < 0, GRAN)
        rows, rids = hs_k[m].astype(np.float64), ids_k[m]
        for cq in np.unique(rids):
            sel = rows[rids == cq]
            bsum[cq] += sel.sum(axis=0)
            bsq[cq] += (sel * sel).sum(axis=0)
    return {"hta": hta, "htb": htb}, gcls, bsum, bsq


def _decode_bn(block):
    """block: [128, 4, 6] f64 -> (sums[128], sumsq[128])."""
    ce, me, ve = block[:, :, 0], block[:, :, 1], block[:, :, 2]
    co, mo, vo = block[:, :, 3], block[:, :, 4], block[:, :, 5]
    sums = (ce * me + co * mo).sum(axis=1)
    sumsq = (ve + ce * me * me + vo + co * mo * mo).sum(axis=1)
    return sums, sumsq


def _device_stats(hidden, ids, **run_kwargs):
    """Returns (sums[C,D], sumsq[C,D]) float64, plus the raw run result."""
    from concourse import bass_utils

    nc = _get_nc()

    order = np.argsort(ids, kind="stable")       # GLOBAL sort by class
    ids_s = ids[order]
    hs = hidden[order]

    in_maps = []
    meta = []
    sums = np.zeros((C, D), dtype=np.float64)
    sumsq = np.zeros((C, D), dtype=np.float64)
    for k in range(NCORES):
        rows = slice(k * ROWS, (k + 1) * ROWS)
        im, gcls, bsum, bsq = _prep_core(hs[rows], ids_s[rows])
        in_maps.append(im)
        meta.append(gcls)
        sums += bsum
        sumsq += bsq

    res = bass_utils.run_bass_kernel_spmd(nc, in_maps, list(range(NCORES)),
                                          **run_kwargs)

    DPE = PEBLK * P  # 384 features on the PE path
    for k in range(NCORES):
        gcls = meta[k]
        peo = res.results[k]["peo"].astype(np.float64)   # [128, 12, 129]
        st = peo.reshape(P, NGRAN, PEBLK, BCOL).transpose(1, 0, 2, 3)
        acc = res.results[k]["acc"].astype(np.float64)   # [128, 124]

        gsums = np.empty((NGRAN, D))
        gsq = np.empty((NGRAN, D))
        # features 0-383 from PE Gram blocks (feature = b*128 + f)
        gsums[:, :DPE] = st[:, :, :, P].transpose(0, 2, 1).reshape(NGRAN, DPE)
        gsq[:, :DPE] = np.diagonal(st[:, :, :, :P], axis1=1,
                                   axis2=3).reshape(NGRAN, DPE)
        # features 384-511: ACT accums for g0, DVE bn for g1-g3
        gsq[0, DPE:] = acc[:, 120]
        gsums[0, DPE:] = acc[:, 121]
        for g in range(1, NGRAN):
            s, sq = _decode_bn(acc[:, 24 * (g - 1):24 * (g - 1) + 24]
                               .reshape(P, 4, 6))
            gsums[g, DPE:] = s
            gsq[g, DPE:] = sq

        for g in range(NGRAN):
            cc = gcls[g]
            if cc >= 0:
                sums[cc] += gsums[g]
                sumsq[cc] += gsq[g]
    return sums, sumsq, res


def _pairwise_loss(counts, sums, sumsq, d):
    """The tiny O(C^2 D) stage on host CPU.

    Runs in float32 with the same jax ops as the reference: at these extreme
    betainc parameters (b ~ 8190, x ~ 1e-5) jax's f32 betainc differs from
    the true (f64) value by ~1e-3, so matching the reference requires
    replicating its f32 numerics, not improving on them.
    """
    import jax
    import jax.numpy as jnp

    cpu = jax.devices("cpu")[0]
    with jax.default_device(cpu):
        counts64 = counts.astype(np.float64)
        means64 = sums / counts64[:, None]
        withins64 = sumsq - counts64[:, None] * means64**2
        counts = jnp.asarray(counts64, jnp.float32)               # [C]
        means = jnp.asarray(means64, jnp.float32)                 # [C, D]
        withins = jnp.asarray(withins64, jnp.float32)             # [C, D]
        half_diff = (means[:, None, :] - means[None, :, :]) * 0.5
        pair_counts = counts[:, None] + counts[None, :]
        pair_between = half_diff * half_diff * pair_counts[:, :, None]
        pair_within = withins[:, None, :] + withins[None, :, :]
        d2 = pair_counts - 2.0
        d2 = jnp.where(d2 == 0.0, 1e-5, d2)
        x = pair_between / (pair_between + pair_within)
        x = jnp.clip(x, XMIN, XMAX)
        a = jnp.full_like(x, 0.5)
        b = jnp.broadcast_to((d2 * 0.5)[:, :, None], x.shape)
        xbetainc = jax.scipy.special.betainc(a, b, x)             # [C, C, D]
        top_k, _ = jax.lax.top_k(xbetainc, int(d))                # [C, C, d]
        per_pair = jnp.sum(jnp.log(top_k), axis=-1)               # [C, C]
        mask = jnp.triu(jnp.ones((C, C), dtype=bool), k=1)
        total = jnp.sum(jnp.where(mask, per_pair, jnp.zeros_like(per_pair)))
        return float(-total)


def kernel(hidden, batch_ids, d):
    hidden = np.asarray(hidden, dtype=np.float32)
    ids = np.asarray(batch_ids).astype(np.int64)
    assert hidden.shape == (N, D), hidden.shape

    counts = np.bincount(ids, minlength=C).astype(np.float64)
    sums, sumsq, _ = _device_stats(hidden, ids)
    total = _pairwise_loss(counts, sums, sumsq, int(np.asarray(d)))
    return np.array(total, dtype=np.float32)
